# revision 27
# baseline (speedup 1.0000x reference)
"""Trainium2 Bass kernel for CategoricalDnn: embedding gather + BatchNorm(train) + ReLU + concat.

Reference computation (B=65536, F=32, V=1001, D=64, N_NUM=16):
    emb[b,f,:]  = tables[f, cat_idx[b,f], :]
    mean/var    = biased batch stats of emb over b (global batch)
    normed      = (emb - mean) * rsqrt(var+eps) * gamma + beta
    out         = concat([relu(normed).reshape(B, F*D), numerical], axis=1)

Strategy (8 NeuronCores, data-parallel over the batch):
  * Batch stats do not need the gathered data: sum_b T[f,idx_b,d] =
    sum_v count[f,v]*T[f,v,d]. The host computes the GLOBAL index histogram
    (integer-only preprocessing of cat_idx); each core contracts it against
    the table on the PE (count^T @ [T | T^2] over v-chunks of 128) to get
    identical global statistics -- no collective, no second data pass.
  * Single gather pass via the ant-ucode dma_gather: 1024 rows x 256B per
    instruction (the SWDGE descriptor ring caps one instruction at 1024
    descriptors; 2048 crashes the runtime). 4 gathers cover one 128-row
    tile across all 32 features -- 8x fewer SWDGE instructions than the
    per-feature indirect-DMA form, which was issue-bound on gpsimd at
    ~1.4us per instruction.
  * Per tile, in a rotating buffer pool: DVE x*scale+shift in place,
    ACT ReLU in place, HWDGE store. Numerical columns are stored once by a
    single strided DMA.
"""

import sys

import numpy as np

if "/opt/trn_rl_repo" not in sys.path:
    sys.path.insert(0, "/opt/trn_rl_repo")

import ml_dtypes

import concourse.bacc as bacc
import concourse.bass as bass
import concourse.mybir as mybir
from concourse.bass_utils import run_bass_kernel_spmd

# Problem constants (hardcoded per harness contract).
B, F, V, D, N_NUM = 65536, 32, 1001, 64, 16
EPS = 1e-5
NCORES = 8
BC = B // NCORES          # 8192 batch rows per core
TILE = 128                # batch rows per gather tile
NT = BC // TILE           # 64 tiles per core
FD = F * D                # 2048
OW = FD + N_NUM           # 2064 output columns
R = F * V                 # 32032 flat table rows (row = v*F + f)
VP = 1024                 # padded vocab (8 chunks of 128)
KC = VP // 128            # 8 v-chunks
P = 128
NGPT = 4                  # dma_gather calls per tile (1024 idxs each)
NG = NT * NGPT            # gathers per core
GC = P * (F // NGPT) // 16  # idx columns per gather
GI = NG * GC              # idx columns per partition (int16)

f32 = mybir.dt.float32
bf16 = mybir.dt.bfloat16
i32 = mybir.dt.int32
i16 = mybir.dt.int16

NP = 4                    # rotating tile buffers


def _build_nc() -> bass.Bass:
    nc = bacc.Bacc("TRN2", target_bir_lowering=False, debug=False,
                   num_devices=NCORES, num_swdge_queues=4,
                   dynamic_dma_scratch_size=65536)

    tabF = nc.dram_tensor("tabF", [R, D], f32, kind="ExternalInput")
    tabRh = nc.dram_tensor("tabRh", [P, KC * FD], bf16, kind="ExternalInput")
    idxh = nc.dram_tensor("idxh", [P, GI], i16, kind="ExternalInput")
    cnth = nc.dram_tensor("cnth", [P, KC * F], bf16, kind="ExternalInput")
    maskh = nc.dram_tensor("maskh", [F, FD], bf16, kind="ExternalInput")
    gbh = nc.dram_tensor("gbh", [2, FD], f32, kind="ExternalInput")
    numh = nc.dram_tensor("numh", [P, NT * N_NUM], f32, kind="ExternalInput")
    out = nc.dram_tensor("out", [BC, OW], f32, kind="ExternalOutput")

    from contextlib import ExitStack
    with ExitStack() as ctx:
        sb = lambda name, shape, dt: ctx.enter_context(
            nc.sbuf_tensor(name, shape, dt))
        idx_sb = sb("idx_sb", [P, GI], i16)
        tabR = sb("tabR", [P, 2 * FD], bf16)  # 2-slot rotating stats chunks
        t2 = sb("t2", [P, FD], bf16)
        cnt_sb = sb("cnt_sb", [P, KC * F], bf16)
        mask_sb = sb("mask_sb", [F, FD], bf16)
        msk_t = sb("msk_t", [F, FD], f32)
        bufs = [sb(f"buf{k}", [P, FD], f32) for k in range(NP)]
        num_sb = sb("num_sb", [P, NT * N_NUM], f32)
        sc_bc = sb("sc_bc", [P, FD], f32)
        sh_bc = sb("sh_bc", [P, FD], f32)
        ga_sb = sb("ga_sb", [1, FD], f32)[:, :]
        be_sb = sb("be_sb", [1, FD], f32)[:, :]
        ds = sb("ds", [1, FD], f32)[:, :]     # diag sum -> mean
        dq = sb("dq", [1, FD], f32)[:, :]     # diag sumsq -> var -> rstd
        srow = sb("srow", [1, FD], f32)[:, :]  # mean^2 tmp -> scale
        trow = sb("trow", [1, FD], f32)[:, :]  # shift
        ones32 = sb("ones32", [F, 1], f32)
        ones_r = sb("ones_r", [1, P], f32)
        eps_row = sb("eps_row", [1, 1], f32)
        wid_sb = sb("wid_sb", [P, P // 16], i16)

        ps_a = ctx.enter_context(nc.psum_tensor("ps_a", [P, FD], f32))
        ps_b = ctx.enter_context(nc.psum_tensor("ps_b", [P, FD], f32))

        sem = lambda name: ctx.enter_context(nc.semaphore(name))
        s_ld = sem("s_ld")
        s_cnt = sem("s_cnt")
        s_msk = sem("s_msk")
        s_gb2 = sem("s_gb2")
        s_lt = [sem("s_lt0"), sem("s_lt1")]
        s_idx = sem("s_idx")
        s_g = [sem(f"s_g{k}") for k in range(NP)]
        s_m = sem("s_m")
        s_v = sem("s_v")
        s_r = sem("s_r")
        s_w = [sem(f"s_w{k}") for k in range(NP)]
        s_num = sem("s_num")
        s_pe = sem("s_pe")
        s_pemq = sem("s_pemq")
        s_sq = sem("s_sq")
        s_tsum = sem("s_tsum")
        s_warm = sem("s_warm")
        s_dve = sem("s_dve")
        s_ax = sem("s_ax")

        # s_ld thresholds after each initial HWDGE load
        LD_CNT, LD_MASK, LD_GA, LD_BE = 16, 32, 48, 64
        LD_TAB = lambda k: 80 + 16 * k  # chunk k loaded

        N_DVE = 14  # s_dve value when sc_bc/sh_bc are ready

        with nc.Block("main") as block:

            @block.sync
            def _(sync):
                H = GI // 8
                sync.dma_start(cnt_sb[:, :], cnth[:, :]).then_inc(s_cnt, 16)
                sync.dma_start(idx_sb[:, 0:H], idxh[:, 0:H]).then_inc(s_idx, 16)
                sync.dma_start(mask_sb[:, :], maskh[:, :]).then_inc(s_msk, 16)
                sync.dma_start(ga_sb, gbh[0:1, :]).then_inc(s_gb2, 16)
                sync.dma_start(be_sb, gbh[1:2, :]).then_inc(s_gb2, 16)
                for k in range(KC):
                    if k >= 2:
                        # slot free when sums-matmuls + square of k-2 done
                        sync.wait_ge(s_tsum, k - 1)
                        sync.wait_ge(s_sq, k - 1)
                    sync.dma_start(
                        tabR[:, (k % 2) * FD:(k % 2 + 1) * FD],
                        tabRh[:, k * FD:(k + 1) * FD],
                    ).then_inc(s_lt[k % 2], 16)
                for h in range(1, 8):
                    sync.dma_start(
                        idx_sb[:, h * H:(h + 1) * H],
                        idxh[:, h * H:(h + 1) * H]).then_inc(s_idx, 16)
                sync.dma_start(num_sb[:, :], numh[:, :]).then_inc(s_ld, 16)
                # numerical columns: per-tile 2-level-AP stores (the 3-level
                # strided form miscompiles on this runtime). Queue entries
                # run concurrently across DMA engines, so an explicit wait
                # on the num load is required before the stores read SBUF.
                sync.wait_ge(s_ld, 16)
                for t in range(NT):
                    sync.dma_start(
                        out[t * TILE:(t + 1) * TILE, FD:OW],
                        num_sb[:, t * N_NUM:(t + 1) * N_NUM],
                    ).then_inc(s_num, 16)
                # per-tile embedding stores
                for t in range(NT):
                    sync.wait_ge(s_r, t + 1)
                    sync.dma_start(
                        out[t * TILE:(t + 1) * TILE, :FD],
                        bufs[t % NP][:, :],
                    ).then_inc(s_w[t % NP], 16)
                for k in range(NP):
                    sync.wait_ge(s_w[k], 16 * ((NT - 1 - k) // NP + 1))
                sync.wait_ge(s_num, 16 * NT)

            @block.gpsimd
            def _(gpsimd):
                for t in range(NT):
                    gpsimd.wait_ge(s_idx, 16 if t < 8 else 16 * 8)
                    if t >= NP:
                        gpsimd.wait_ge(s_w[t % NP], 16 * (t // NP))
                    buf3 = bufs[t % NP][:, :].rearrange(
                        "p (f d) -> p f d", d=D)
                    for q in range(NGPT):
                        g = t * NGPT + q
                        gpsimd.dma_gather(
                            out_ap=buf3[:, q * (F // NGPT):(q + 1) * (F // NGPT), :],
                            in_ap=tabF[:, :],
                            idxs_ap=idx_sb[:, g * GC:(g + 1) * GC],
                            num_idxs=P * (F // NGPT),
                            num_idxs_reg=P * (F // NGPT),
                            elem_size=D,
                            queue_num=q % 4,
                        ).then_inc(s_g[t % NP], 16)

            @block.tensor
            def _(tensor):
                tensor.wait_ge(s_cnt, 16)
                for k in range(KC):
                    tensor.wait_ge(s_lt[k % 2], 16 * (k // 2 + 1))
                    for j in range(4):
                        mm = tensor.matmul(
                            ps_a[0:F, j * 512:(j + 1) * 512],
                            cnt_sb[:, k * F:(k + 1) * F],
                            tabR[:, (k % 2) * FD + j * 512:(k % 2) * FD + (j + 1) * 512],
                            start=(k == 0), stop=(k == KC - 1),
                            skip_group_check=True)
                    if k == KC - 1:
                        mm.then_inc(s_pe, 1)           # s_pe=1: sums done
                    else:
                        mm.then_inc(s_tsum, 1)         # slot consumed by sums
                    tensor.wait_ge(s_sq, k + 1)
                    for j in range(4):
                        mm = tensor.matmul(
                            ps_b[0:F, j * 512:(j + 1) * 512],
                            cnt_sb[:, k * F:(k + 1) * F],
                            t2[:, j * 512:(j + 1) * 512],
                            start=(k == 0), stop=(k == KC - 1),
                            skip_group_check=True)
                    mm.then_inc(s_pemq, 1)   # t2 free for chunk k+1; ==8: sq done
                # diag extraction colsums (masked rows live in msk_t[0:F])
                tensor.wait_ge(s_dve, 1)
                for j in range(4):
                    mm = tensor.matmul(
                        ps_a[0:1, j * 512:(j + 1) * 512], ones32[:, :],
                        msk_t[:, j * 512:(j + 1) * 512],
                        start=True, stop=True, skip_group_check=True)
                mm.then_inc(s_pe, 1)                   # s_pe=2: diag_s in ps_a[0]
                tensor.wait_ge(s_dve, 3)
                for j in range(4):
                    mm = tensor.matmul(
                        ps_b[0:1, j * 512:(j + 1) * 512], ones32[:, :],
                        msk_t[:, j * 512:(j + 1) * 512],
                        start=True, stop=True, skip_group_check=True)
                mm.then_inc(s_pe, 1)                   # s_pe=3: diag_q in ps_b[0]
                # broadcast scale/shift rows to 128 partitions
                tensor.wait_ge(s_dve, 10)
                for j in range(4):
                    mm = tensor.matmul(
                        ps_a[:, j * 512:(j + 1) * 512], ones_r[:, :],
                        srow[:, j * 512:(j + 1) * 512],
                        start=True, stop=True, skip_group_check=True)
                mm.then_inc(s_pe, 1)                   # s_pe=4: scale bcast
                tensor.wait_ge(s_dve, 12)
                for j in range(4):
                    mm = tensor.matmul(
                        ps_b[:, j * 512:(j + 1) * 512], ones_r[:, :],
                        trow[:, j * 512:(j + 1) * 512],
                        start=True, stop=True, skip_group_check=True)
                mm.then_inc(s_pe, 1)                   # s_pe=5: shift bcast

            @block.scalar
            def _(scalar):
                for k in range(KC):
                    scalar.wait_ge(s_lt[k % 2], 16 * (k // 2 + 1))
                    if k >= 1:
                        scalar.wait_ge(s_pemq, k)      # t2 consumed
                    scalar.square(
                        t2[:, :], tabR[:, (k % 2) * FD:(k % 2 + 1) * FD],
                    ).then_inc(s_sq, 1)
                # sqrt(var + eps) on the dq row
                scalar.wait_ge(s_dve, 8)
                scalar.activation(
                    dq, dq, mybir.ActivationFunctionType.Sqrt,
                    bias=eps_row[:, :],
                ).then_inc(s_ax, 1)
                # per-tile relu (in place)
                for t in range(NT):
                    scalar.wait_ge(s_v, t + 1)
                    scalar.activation(
                        bufs[t % NP][:, :], bufs[t % NP][:, :],
                        mybir.ActivationFunctionType.Relu,
                    ).then_inc(s_r, 1)

            @block.vector
            def _(vector):
                vector.memset(ones32[:, :], 1.0)
                vector.memset(ones_r[:, :], 1.0)
                vector.memset(eps_row[:, :], float(EPS))  # all 4 partitions
                # ---- stats rows: every DVE op self-chained via s_dve ----
                vsn = [0]

                def vstep(emit, *waits):
                    vector.wait_ge(s_dve, vsn[0])
                    for w_sem, w_val in waits:
                        vector.wait_ge(w_sem, w_val)
                    inst = emit()
                    vsn[0] += 1
                    inst.then_inc(s_dve, 1)
                    return inst

                vstep(lambda: vector.tensor_mul(                    # s_dve=1
                    msk_t[:, :], ps_a[0:F, :], mask_sb[:, :]),
                    (s_pe, 1), (s_msk, 16))
                vstep(lambda: vector.tensor_copy(ds, ps_a[0:1, :]),
                      (s_pe, 2))                                    # s_dve=2
                vstep(lambda: vector.tensor_mul(                    # s_dve=3
                    msk_t[:, :], ps_b[0:F, :], mask_sb[:, :]),
                    (s_pe, 2), (s_pemq, KC))
                vstep(lambda: vector.tensor_copy(dq, ps_b[0:1, :]),
                      (s_pe, 3))                                    # s_dve=4
                vstep(lambda: vector.tensor_scalar_mul(
                    ds, ds, 1.0 / B))                   # s_dve=5 mean
                vstep(lambda: vector.tensor_scalar_mul(
                    dq, dq, 1.0 / B))                   # s_dve=6 E[x^2]
                vstep(lambda: vector.tensor_mul(
                    srow, ds, ds))                # s_dve=7 mean^2
                vstep(lambda: vector.tensor_sub(
                    dq, dq, srow))                # s_dve=8 var
                vstep(lambda: vector.reciprocal_approx_fast(dq, dq),
                      (s_ax, 1))                                    # s_dve=9 rstd
                vstep(lambda: vector.tensor_mul(
                    srow, ga_sb, dq),
                    (s_gb2, 32))                                  # s_dve=10 scale
                vstep(lambda: vector.tensor_mul(
                    trow, ds, srow))              # s_dve=11
                vstep(lambda: vector.tensor_sub(
                    trow, be_sb, trow),
                    (s_gb2, 32))                                  # s_dve=12 shift
                vstep(lambda: vector.tensor_copy(sc_bc[:, :], ps_a[:, :]),
                      (s_pe, 4))                                    # s_dve=13
                vstep(lambda: vector.tensor_copy(sh_bc[:, :], ps_b[:, :]),
                      (s_pe, 5))                                    # s_dve=14
                assert vsn[0] == N_DVE
                # ---- per-tile normalize (in place) ----
                for t in range(NT):
                    vector.wait_ge(s_dve, N_DVE)
                    vector.wait_ge(s_g[t % NP], 16 * NGPT * (t // NP + 1))
                    vector.tensor_mul(
                        bufs[t % NP][:, :], bufs[t % NP][:, :],
                        sc_bc[:, :]).then_inc(s_m, 1)
                    vector.wait_ge(s_m, t + 1)
                    vector.tensor_add(
                        bufs[t % NP][:, :], bufs[t % NP][:, :],
                        sh_bc[:, :]).then_inc(s_v, 1)

        nc.compile()
    return nc


_NC_CACHE: list = []

# Optional profiling knobs (used by test harnesses; harmless defaults).
TRACE = False
TMPDIR = None
LAST_RESULT: list = []


def _get_nc():
    if not _NC_CACHE:
        _NC_CACHE.append(_build_nc())
    return _NC_CACHE[0]


def _host_prep(cat_idx, numerical, tables, gamma, beta):
    """Host-side layout/preprocessing (indices + replication only)."""
    # linear gather rows: row = v*F + f  (max 32031, fits int16)
    lin = (cat_idx.astype(np.int32) * F
           + np.arange(F, dtype=np.int32)[None, :])          # [B, F]

    # dma_gather idx layout. Gather g = t*NGPT + q covers logical indices
    # i = f_local*128 + p (f = q*8 + f_local, batch row = t*128 + p); the
    # ucode reads logical index i from partition i%16, column g*GC + i//16,
    # replicated to all 8 16-partition groups.
    i_grid = np.arange(P * (F // NGPT))                       # [1024]
    f_loc = i_grid >> 7                                       # i // 128
    p_ = i_grid & 127                                         # i % 128
    ip = (i_grid % 16)                                        # partition
    jc = (i_grid // 16)                                       # column in gather
    lin16 = lin.astype(np.int16).reshape(NCORES, NT, TILE, NGPT, F // NGPT)
    idx_pc = np.zeros((NCORES, 16, GI), dtype=np.int16)
    for q in range(NGPT):
        # vals[c, t, i] = lin16[c, t, p_[i], q, f_loc[i]]
        vals = lin16[:, :, :, q, :][:, :, p_, f_loc]          # [NC, NT, 1024]
        col = (np.arange(NT)[:, None] * NGPT + q) * GC + jc[None, :]
        idx_pc[:, ip[None, :].repeat(NT, 0), col] = vals
    idx_pc = np.ascontiguousarray(
        np.tile(idx_pc, (1, 8, 1)))                           # [NC, 128, GI]

    # global histogram over linear rows (integer-only preprocessing)
    cnt = np.bincount(lin.ravel(), minlength=VP * F).reshape(VP, F)
    cnt_in = np.ascontiguousarray(
        cnt.reshape(KC, TILE, F).transpose(1, 0, 2).reshape(P, KC * F)
    ).astype(ml_dtypes.bfloat16)

    # flat gather table [R, D] f32, row = v*F + f
    tabF = np.ascontiguousarray(
        tables.transpose(1, 0, 2).reshape(R, D)).astype(np.float32)

    # stats table layout [128, k*2048 + f*64 + d] = T[f, k*128+p, d], bf16
    tpad = np.zeros((F, VP, D), dtype=np.float32)
    tpad[:, :V] = tables
    tabR = np.ascontiguousarray(
        tpad.reshape(F, KC, TILE, D).transpose(2, 1, 0, 3).reshape(P, KC * FD)
    ).astype(ml_dtypes.bfloat16)

    mask = np.zeros((F, FD), dtype=np.float32)
    for f in range(F):
        mask[f, f * D:(f + 1) * D] = 1.0
    mask = mask.astype(ml_dtypes.bfloat16)

    gb = np.ascontiguousarray(
        np.stack([gamma.reshape(FD), beta.reshape(FD)], axis=0))

    num_pc = []
    for c in range(NCORES):
        sh = numerical[c * BC:(c + 1) * BC].reshape(NT, P, N_NUM)
        num_pc.append(np.ascontiguousarray(
            sh.transpose(1, 0, 2).reshape(P, NT * N_NUM)))

    return idx_pc, cnt_in, tabF, tabR, mask, gb, num_pc


def kernel(cat_idx, numerical, tables, gamma, beta):
    cat_idx = np.asarray(cat_idx)
    numerical = np.asarray(numerical, dtype=np.float32)
    tables = np.asarray(tables, dtype=np.float32)
    gamma = np.asarray(gamma, dtype=np.float32)
    beta = np.asarray(beta, dtype=np.float32)

    nc = _get_nc()
    idx_pc, cnt_in, tabF, tabR, mask, gb, num_pc = _host_prep(
        cat_idx, numerical, tables, gamma, beta)

    in_maps = [
        {"tabF": tabF, "tabRh": tabR, "idxh": idx_pc[c], "cnth": cnt_in,
         "maskh": mask, "gbh": gb, "numh": num_pc[c]}
        for c in range(NCORES)
    ]
    res = run_bass_kernel_spmd(nc, in_maps, core_ids=list(range(NCORES)),
                               trace=TRACE, tmpdir=TMPDIR)
    LAST_RESULT.clear()
    LAST_RESULT.append(res)
    out = np.concatenate([res.results[c]["out"] for c in range(NCORES)], axis=0)
    return out



# revision 28
# speedup vs baseline: 1.0255x; 1.0255x over previous
"""Trainium2 Bass kernel for CategoricalDnn: embedding gather + BatchNorm(train) + ReLU + concat.

Reference computation (B=65536, F=32, V=1001, D=64, N_NUM=16):
    emb[b,f,:]  = tables[f, cat_idx[b,f], :]
    mean/var    = biased batch stats of emb over b (global batch)
    normed      = (emb - mean) * rsqrt(var+eps) * gamma + beta
    out         = concat([relu(normed).reshape(B, F*D), numerical], axis=1)

Strategy (8 NeuronCores, data-parallel over the batch):
  * Batch stats do not need the gathered data: sum_b T[f,idx_b,d] =
    sum_v count[f,v]*T[f,v,d]. The host computes the GLOBAL index histogram
    (integer-only preprocessing of cat_idx); each core contracts it against
    the table on the PE (count^T @ [T | T^2] over v-chunks of 128) to get
    identical global statistics -- no collective, no second data pass.
  * Single gather pass via the ant-ucode dma_gather: 1024 rows x 256B per
    instruction (the SWDGE descriptor ring caps one instruction at 1024
    descriptors; 2048 crashes the runtime). 4 gathers cover one 128-row
    tile across all 32 features -- 8x fewer SWDGE instructions than the
    per-feature indirect-DMA form, which was issue-bound on gpsimd at
    ~1.4us per instruction.
  * Per tile, in a rotating buffer pool: DVE x*scale+shift in place,
    ACT ReLU in place, HWDGE store. Numerical columns are stored once by a
    single strided DMA.
"""

import sys

import numpy as np

if "/opt/trn_rl_repo" not in sys.path:
    sys.path.insert(0, "/opt/trn_rl_repo")

import ml_dtypes

import concourse.bacc as bacc
import concourse.bass as bass
import concourse.mybir as mybir
from concourse.bass_utils import run_bass_kernel_spmd

# Problem constants (hardcoded per harness contract).
B, F, V, D, N_NUM = 65536, 32, 1001, 64, 16
EPS = 1e-5
NCORES = 8
BC = B // NCORES          # 8192 batch rows per core
TILE = 128                # batch rows per gather tile
NT = BC // TILE           # 64 tiles per core
FD = F * D                # 2048
OW = FD + N_NUM           # 2064 output columns
R = F * V                 # 32032 flat table rows (row = v*F + f)
VP = 1024                 # padded vocab (8 chunks of 128)
KC = VP // 128            # 8 v-chunks
P = 128
NGPT = 4                  # dma_gather calls per tile (1024 idxs each)
NG = NT * NGPT            # gathers per core
GC = P * (F // NGPT) // 16  # idx columns per gather
GI = NG * GC              # idx columns per partition (int16)

f32 = mybir.dt.float32
bf16 = mybir.dt.bfloat16
i32 = mybir.dt.int32
i16 = mybir.dt.int16

NP = 4                    # rotating tile buffers


def _build_nc() -> bass.Bass:
    nc = bacc.Bacc("TRN2", target_bir_lowering=False, debug=False,
                   num_devices=NCORES, num_swdge_queues=4,
                   dynamic_dma_scratch_size=65536)

    tabF = nc.dram_tensor("tabF", [R, D], f32, kind="ExternalInput")
    tabRh = nc.dram_tensor("tabRh", [P, KC * FD], bf16, kind="ExternalInput")
    idxh = nc.dram_tensor("idxh", [P, GI], i16, kind="ExternalInput")
    cnth = nc.dram_tensor("cnth", [P, KC * F], bf16, kind="ExternalInput")
    maskh = nc.dram_tensor("maskh", [F, FD], bf16, kind="ExternalInput")
    gbh = nc.dram_tensor("gbh", [2, FD], f32, kind="ExternalInput")
    numh = nc.dram_tensor("numh", [P, NT * N_NUM], f32, kind="ExternalInput")
    out = nc.dram_tensor("out", [BC, OW], f32, kind="ExternalOutput")

    from contextlib import ExitStack
    with ExitStack() as ctx:
        sb = lambda name, shape, dt: ctx.enter_context(
            nc.sbuf_tensor(name, shape, dt))
        idx_sb = sb("idx_sb", [P, GI], i16)
        tabR = sb("tabR", [P, 2 * FD], bf16)  # 2-slot rotating stats chunks
        t2 = sb("t2", [P, FD], bf16)
        cnt_sb = sb("cnt_sb", [P, KC * F], bf16)
        mask_sb = sb("mask_sb", [F, FD], bf16)
        msk_t = sb("msk_t", [F, FD], f32)
        bufs = [sb(f"buf{k}", [P, FD], f32) for k in range(NP)]
        num_sb = sb("num_sb", [P, NT * N_NUM], f32)
        sc_bc = sb("sc_bc", [P, FD], f32)
        sh_bc = sb("sh_bc", [P, FD], f32)
        ga_sb = sb("ga_sb", [1, FD], f32)[:, :]
        be_sb = sb("be_sb", [1, FD], f32)[:, :]
        ds = sb("ds", [1, FD], f32)[:, :]     # diag sum -> mean
        dq = sb("dq", [1, FD], f32)[:, :]     # diag sumsq -> var -> rstd
        srow = sb("srow", [1, FD], f32)[:, :]  # mean^2 tmp -> scale
        trow = sb("trow", [1, FD], f32)[:, :]  # shift
        ones32 = sb("ones32", [F, 1], f32)
        ones_r = sb("ones_r", [1, P], f32)
        eps_row = sb("eps_row", [1, 1], f32)
        wid_sb = sb("wid_sb", [P, P // 16], i16)

        ps_a = ctx.enter_context(nc.psum_tensor("ps_a", [P, FD], f32))
        ps_b = ctx.enter_context(nc.psum_tensor("ps_b", [P, FD], f32))

        sem = lambda name: ctx.enter_context(nc.semaphore(name))
        s_ld = sem("s_ld")
        s_cnt = sem("s_cnt")
        s_msk = sem("s_msk")
        s_gb2 = sem("s_gb2")
        s_lt = [sem("s_lt0"), sem("s_lt1")]
        s_idx = sem("s_idx")
        s_g = [sem(f"s_g{k}") for k in range(NP)]
        s_m = sem("s_m")
        s_v = sem("s_v")
        s_r = sem("s_r")
        s_w = [sem(f"s_w{k}") for k in range(NP)]
        s_num = sem("s_num")
        s_pe = sem("s_pe")
        s_pemq = sem("s_pemq")
        s_sq = sem("s_sq")
        s_tsum = sem("s_tsum")
        s_warm = sem("s_warm")
        s_dve = sem("s_dve")
        s_ax = sem("s_ax")

        # s_ld thresholds after each initial HWDGE load
        LD_CNT, LD_MASK, LD_GA, LD_BE = 16, 32, 48, 64
        LD_TAB = lambda k: 80 + 16 * k  # chunk k loaded

        N_DVE = 14  # s_dve value when sc_bc/sh_bc are ready

        with nc.Block("main") as block:

            @block.sync
            def _(sync):
                H = GI // 8
                sync.dma_start(cnt_sb[:, :], cnth[:, :]).then_inc(s_cnt, 16)
                sync.dma_start(idx_sb[:, 0:H], idxh[:, 0:H]).then_inc(s_idx, 16)
                sync.dma_start(mask_sb[:, :], maskh[:, :]).then_inc(s_msk, 16)
                sync.dma_start(ga_sb, gbh[0:1, :]).then_inc(s_gb2, 16)
                sync.dma_start(be_sb, gbh[1:2, :]).then_inc(s_gb2, 16)
                for k in range(KC):
                    if k >= 2:
                        # slot free when sums-matmuls + square of k-2 done
                        sync.wait_ge(s_tsum, k - 1)
                        sync.wait_ge(s_sq, k - 1)
                    sync.dma_start(
                        tabR[:, (k % 2) * FD:(k % 2 + 1) * FD],
                        tabRh[:, k * FD:(k + 1) * FD],
                    ).then_inc(s_lt[k % 2], 16)
                sync.dma_start(num_sb[:, :], numh[:, :]).then_inc(s_ld, 16)
                for h in range(1, 8):
                    sync.dma_start(
                        idx_sb[:, h * H:(h + 1) * H],
                        idxh[:, h * H:(h + 1) * H]).then_inc(s_idx, 16)
                # numerical columns: per-tile 2-level-AP stores (the 3-level
                # strided form miscompiles on this runtime), interleaved with
                # the embedding stores so their issue cost hides in the
                # inter-tile gaps. s_ld==16 is exact: num is its only load.
                sync.wait_ge(s_ld, 16)
                for t in range(NT):
                    sync.wait_ge(s_r, t + 1)
                    sync.dma_start(
                        out[t * TILE:(t + 1) * TILE, :FD],
                        bufs[t % NP][:, :],
                    ).then_inc(s_w[t % NP], 16)
                    sync.dma_start(
                        out[t * TILE:(t + 1) * TILE, FD:OW],
                        num_sb[:, t * N_NUM:(t + 1) * N_NUM],
                    ).then_inc(s_num, 16)
                for k in range(NP):
                    sync.wait_ge(s_w[k], 16 * ((NT - 1 - k) // NP + 1))
                sync.wait_ge(s_num, 16 * NT)

            @block.gpsimd
            def _(gpsimd):
                for t in range(NT):
                    gpsimd.wait_ge(s_idx, 16 if t < 8 else 16 * 8)
                    if t >= NP:
                        gpsimd.wait_ge(s_w[t % NP], 16 * (t // NP))
                    buf3 = bufs[t % NP][:, :].rearrange(
                        "p (f d) -> p f d", d=D)
                    for q in range(NGPT):
                        g = t * NGPT + q
                        gpsimd.dma_gather(
                            out_ap=buf3[:, q * (F // NGPT):(q + 1) * (F // NGPT), :],
                            in_ap=tabF[:, :],
                            idxs_ap=idx_sb[:, g * GC:(g + 1) * GC],
                            num_idxs=P * (F // NGPT),
                            num_idxs_reg=P * (F // NGPT),
                            elem_size=D,
                            queue_num=q % 4,
                        ).then_inc(s_g[t % NP], 16)

            @block.tensor
            def _(tensor):
                tensor.wait_ge(s_cnt, 16)
                for k in range(KC):
                    tensor.wait_ge(s_lt[k % 2], 16 * (k // 2 + 1))
                    for j in range(4):
                        mm = tensor.matmul(
                            ps_a[0:F, j * 512:(j + 1) * 512],
                            cnt_sb[:, k * F:(k + 1) * F],
                            tabR[:, (k % 2) * FD + j * 512:(k % 2) * FD + (j + 1) * 512],
                            start=(k == 0), stop=(k == KC - 1),
                            skip_group_check=True)
                    if k == KC - 1:
                        mm.then_inc(s_pe, 1)           # s_pe=1: sums done
                    else:
                        mm.then_inc(s_tsum, 1)         # slot consumed by sums
                    tensor.wait_ge(s_sq, k + 1)
                    for j in range(4):
                        mm = tensor.matmul(
                            ps_b[0:F, j * 512:(j + 1) * 512],
                            cnt_sb[:, k * F:(k + 1) * F],
                            t2[:, j * 512:(j + 1) * 512],
                            start=(k == 0), stop=(k == KC - 1),
                            skip_group_check=True)
                    mm.then_inc(s_pemq, 1)   # t2 free for chunk k+1; ==8: sq done
                # diag extraction colsums (masked rows live in msk_t[0:F])
                tensor.wait_ge(s_dve, 1)
                for j in range(4):
                    mm = tensor.matmul(
                        ps_a[0:1, j * 512:(j + 1) * 512], ones32[:, :],
                        msk_t[:, j * 512:(j + 1) * 512],
                        start=True, stop=True, skip_group_check=True)
                mm.then_inc(s_pe, 1)                   # s_pe=2: diag_s in ps_a[0]
                tensor.wait_ge(s_dve, 3)
                for j in range(4):
                    mm = tensor.matmul(
                        ps_b[0:1, j * 512:(j + 1) * 512], ones32[:, :],
                        msk_t[:, j * 512:(j + 1) * 512],
                        start=True, stop=True, skip_group_check=True)
                mm.then_inc(s_pe, 1)                   # s_pe=3: diag_q in ps_b[0]
                # broadcast scale/shift rows to 128 partitions
                tensor.wait_ge(s_dve, 10)
                for j in range(4):
                    mm = tensor.matmul(
                        ps_a[:, j * 512:(j + 1) * 512], ones_r[:, :],
                        srow[:, j * 512:(j + 1) * 512],
                        start=True, stop=True, skip_group_check=True)
                mm.then_inc(s_pe, 1)                   # s_pe=4: scale bcast
                tensor.wait_ge(s_dve, 12)
                for j in range(4):
                    mm = tensor.matmul(
                        ps_b[:, j * 512:(j + 1) * 512], ones_r[:, :],
                        trow[:, j * 512:(j + 1) * 512],
                        start=True, stop=True, skip_group_check=True)
                mm.then_inc(s_pe, 1)                   # s_pe=5: shift bcast

            @block.scalar
            def _(scalar):
                for k in range(KC):
                    scalar.wait_ge(s_lt[k % 2], 16 * (k // 2 + 1))
                    if k >= 1:
                        scalar.wait_ge(s_pemq, k)      # t2 consumed
                    scalar.square(
                        t2[:, :], tabR[:, (k % 2) * FD:(k % 2 + 1) * FD],
                    ).then_inc(s_sq, 1)
                # sqrt(var + eps) on the dq row
                scalar.wait_ge(s_dve, 8)
                scalar.activation(
                    dq, dq, mybir.ActivationFunctionType.Sqrt,
                    bias=eps_row[:, :],
                ).then_inc(s_ax, 1)
                # per-tile relu (in place)
                for t in range(NT):
                    scalar.wait_ge(s_v, t + 1)
                    scalar.activation(
                        bufs[t % NP][:, :], bufs[t % NP][:, :],
                        mybir.ActivationFunctionType.Relu,
                    ).then_inc(s_r, 1)

            @block.vector
            def _(vector):
                vector.memset(ones32[:, :], 1.0)
                vector.memset(ones_r[:, :], 1.0)
                vector.memset(eps_row[:, :], float(EPS))  # all 4 partitions
                # ---- stats rows: every DVE op self-chained via s_dve ----
                vsn = [0]

                def vstep(emit, *waits):
                    vector.wait_ge(s_dve, vsn[0])
                    for w_sem, w_val in waits:
                        vector.wait_ge(w_sem, w_val)
                    inst = emit()
                    vsn[0] += 1
                    inst.then_inc(s_dve, 1)
                    return inst

                vstep(lambda: vector.tensor_mul(                    # s_dve=1
                    msk_t[:, :], ps_a[0:F, :], mask_sb[:, :]),
                    (s_pe, 1), (s_msk, 16))
                vstep(lambda: vector.tensor_copy(ds, ps_a[0:1, :]),
                      (s_pe, 2))                                    # s_dve=2
                vstep(lambda: vector.tensor_mul(                    # s_dve=3
                    msk_t[:, :], ps_b[0:F, :], mask_sb[:, :]),
                    (s_pe, 2), (s_pemq, KC))
                vstep(lambda: vector.tensor_copy(dq, ps_b[0:1, :]),
                      (s_pe, 3))                                    # s_dve=4
                vstep(lambda: vector.tensor_scalar_mul(
                    ds, ds, 1.0 / B))                   # s_dve=5 mean
                vstep(lambda: vector.tensor_scalar_mul(
                    dq, dq, 1.0 / B))                   # s_dve=6 E[x^2]
                vstep(lambda: vector.tensor_mul(
                    srow, ds, ds))                # s_dve=7 mean^2
                vstep(lambda: vector.tensor_sub(
                    dq, dq, srow))                # s_dve=8 var
                vstep(lambda: vector.reciprocal_approx_fast(dq, dq),
                      (s_ax, 1))                                    # s_dve=9 rstd
                vstep(lambda: vector.tensor_mul(
                    srow, ga_sb, dq),
                    (s_gb2, 32))                                  # s_dve=10 scale
                vstep(lambda: vector.tensor_mul(
                    trow, ds, srow))              # s_dve=11
                vstep(lambda: vector.tensor_sub(
                    trow, be_sb, trow),
                    (s_gb2, 32))                                  # s_dve=12 shift
                vstep(lambda: vector.tensor_copy(sc_bc[:, :], ps_a[:, :]),
                      (s_pe, 4))                                    # s_dve=13
                vstep(lambda: vector.tensor_copy(sh_bc[:, :], ps_b[:, :]),
                      (s_pe, 5))                                    # s_dve=14
                assert vsn[0] == N_DVE
                # ---- per-tile normalize (in place) ----
                for t in range(NT):
                    vector.wait_ge(s_dve, N_DVE)
                    vector.wait_ge(s_g[t % NP], 16 * NGPT * (t // NP + 1))
                    vector.tensor_mul(
                        bufs[t % NP][:, :], bufs[t % NP][:, :],
                        sc_bc[:, :]).then_inc(s_m, 1)
                    vector.wait_ge(s_m, t + 1)
                    vector.tensor_add(
                        bufs[t % NP][:, :], bufs[t % NP][:, :],
                        sh_bc[:, :]).then_inc(s_v, 1)

        nc.compile()
    return nc


_NC_CACHE: list = []

# Optional profiling knobs (used by test harnesses; harmless defaults).
TRACE = False
TMPDIR = None
LAST_RESULT: list = []


def _get_nc():
    if not _NC_CACHE:
        _NC_CACHE.append(_build_nc())
    return _NC_CACHE[0]


def _host_prep(cat_idx, numerical, tables, gamma, beta):
    """Host-side layout/preprocessing (indices + replication only)."""
    # linear gather rows: row = v*F + f  (max 32031, fits int16)
    lin = (cat_idx.astype(np.int32) * F
           + np.arange(F, dtype=np.int32)[None, :])          # [B, F]

    # dma_gather idx layout. Gather g = t*NGPT + q covers logical indices
    # i = f_local*128 + p (f = q*8 + f_local, batch row = t*128 + p); the
    # ucode reads logical index i from partition i%16, column g*GC + i//16,
    # replicated to all 8 16-partition groups.
    i_grid = np.arange(P * (F // NGPT))                       # [1024]
    f_loc = i_grid >> 7                                       # i // 128
    p_ = i_grid & 127                                         # i % 128
    ip = (i_grid % 16)                                        # partition
    jc = (i_grid // 16)                                       # column in gather
    lin16 = lin.astype(np.int16).reshape(NCORES, NT, TILE, NGPT, F // NGPT)
    idx_pc = np.zeros((NCORES, 16, GI), dtype=np.int16)
    for q in range(NGPT):
        # vals[c, t, i] = lin16[c, t, p_[i], q, f_loc[i]]
        vals = lin16[:, :, :, q, :][:, :, p_, f_loc]          # [NC, NT, 1024]
        col = (np.arange(NT)[:, None] * NGPT + q) * GC + jc[None, :]
        idx_pc[:, ip[None, :].repeat(NT, 0), col] = vals
    idx_pc = np.ascontiguousarray(
        np.tile(idx_pc, (1, 8, 1)))                           # [NC, 128, GI]

    # global histogram over linear rows (integer-only preprocessing)
    cnt = np.bincount(lin.ravel(), minlength=VP * F).reshape(VP, F)
    cnt_in = np.ascontiguousarray(
        cnt.reshape(KC, TILE, F).transpose(1, 0, 2).reshape(P, KC * F)
    ).astype(ml_dtypes.bfloat16)

    # flat gather table [R, D] f32, row = v*F + f
    tabF = np.ascontiguousarray(
        tables.transpose(1, 0, 2).reshape(R, D)).astype(np.float32)

    # stats table layout [128, k*2048 + f*64 + d] = T[f, k*128+p, d], bf16
    tpad = np.zeros((F, VP, D), dtype=np.float32)
    tpad[:, :V] = tables
    tabR = np.ascontiguousarray(
        tpad.reshape(F, KC, TILE, D).transpose(2, 1, 0, 3).reshape(P, KC * FD)
    ).astype(ml_dtypes.bfloat16)

    mask = np.zeros((F, FD), dtype=np.float32)
    for f in range(F):
        mask[f, f * D:(f + 1) * D] = 1.0
    mask = mask.astype(ml_dtypes.bfloat16)

    gb = np.ascontiguousarray(
        np.stack([gamma.reshape(FD), beta.reshape(FD)], axis=0))

    num_pc = []
    for c in range(NCORES):
        sh = numerical[c * BC:(c + 1) * BC].reshape(NT, P, N_NUM)
        num_pc.append(np.ascontiguousarray(
            sh.transpose(1, 0, 2).reshape(P, NT * N_NUM)))

    return idx_pc, cnt_in, tabF, tabR, mask, gb, num_pc


def kernel(cat_idx, numerical, tables, gamma, beta):
    cat_idx = np.asarray(cat_idx)
    numerical = np.asarray(numerical, dtype=np.float32)
    tables = np.asarray(tables, dtype=np.float32)
    gamma = np.asarray(gamma, dtype=np.float32)
    beta = np.asarray(beta, dtype=np.float32)

    nc = _get_nc()
    idx_pc, cnt_in, tabF, tabR, mask, gb, num_pc = _host_prep(
        cat_idx, numerical, tables, gamma, beta)

    in_maps = [
        {"tabF": tabF, "tabRh": tabR, "idxh": idx_pc[c], "cnth": cnt_in,
         "maskh": mask, "gbh": gb, "numh": num_pc[c]}
        for c in range(NCORES)
    ]
    res = run_bass_kernel_spmd(nc, in_maps, core_ids=list(range(NCORES)),
                               trace=TRACE, tmpdir=TMPDIR)
    LAST_RESULT.clear()
    LAST_RESULT.append(res)
    out = np.concatenate([res.results[c]["out"] for c in range(NCORES)], axis=0)
    return out



# revision 30
# speedup vs baseline: 1.0538x; 1.0276x over previous
"""Trainium2 Bass kernel for CategoricalDnn: embedding gather + BatchNorm(train) + ReLU + concat.

Reference computation (B=65536, F=32, V=1001, D=64, N_NUM=16):
    emb[b,f,:]  = tables[f, cat_idx[b,f], :]
    mean/var    = biased batch stats of emb over b (global batch)
    normed      = (emb - mean) * rsqrt(var+eps) * gamma + beta
    out         = concat([relu(normed).reshape(B, F*D), numerical], axis=1)

Strategy (8 NeuronCores, data-parallel over the batch):
  * Batch stats do not need the gathered data: sum_b T[f,idx_b,d] =
    sum_v count[f,v]*T[f,v,d]. The host computes the GLOBAL index histogram
    (integer-only preprocessing of cat_idx); each core contracts it against
    the table on the PE (count^T @ [T | T^2] over v-chunks of 128) to get
    identical global statistics -- no collective, no second data pass.
  * Single gather pass via the ant-ucode dma_gather: 1024 rows x 256B per
    instruction (the SWDGE descriptor ring caps one instruction at 1024
    descriptors; 2048 crashes the runtime). 4 gathers cover one 128-row
    tile across all 32 features -- 8x fewer SWDGE instructions than the
    per-feature indirect-DMA form, which was issue-bound on gpsimd at
    ~1.4us per instruction.
  * Per tile, in a rotating buffer pool: DVE x*scale+shift in place,
    ACT ReLU in place, HWDGE store. Numerical columns are stored once by a
    single strided DMA.
"""

import sys

import numpy as np

if "/opt/trn_rl_repo" not in sys.path:
    sys.path.insert(0, "/opt/trn_rl_repo")

import ml_dtypes

import concourse.bacc as bacc
import concourse.bass as bass
import concourse.mybir as mybir
from concourse.bass_utils import run_bass_kernel_spmd

# Problem constants (hardcoded per harness contract).
B, F, V, D, N_NUM = 65536, 32, 1001, 64, 16
EPS = 1e-5
NCORES = 8
BC = B // NCORES          # 8192 batch rows per core
TILE = 128                # batch rows per gather tile
NT = BC // TILE           # 64 tiles per core
FD = F * D                # 2048
OW = FD + N_NUM           # 2064 output columns
R = F * V                 # 32032 flat table rows (row = v*F + f)
VP = 1024                 # padded vocab (8 chunks of 128)
KC = VP // 128            # 8 v-chunks
P = 128
NGPT = 4                  # dma_gather calls per tile (1024 idxs each)
NG = NT * NGPT            # gathers per core
GC = P * (F // NGPT) // 16  # idx columns per gather
GI = NT * GC              # idx columns per partition (int16; per-queue data in its pair's partitions)

f32 = mybir.dt.float32
bf16 = mybir.dt.bfloat16
i32 = mybir.dt.int32
i16 = mybir.dt.int16

NP = 6                    # rotating tile buffers


def _build_nc() -> bass.Bass:
    nc = bacc.Bacc("TRN2", target_bir_lowering=False, debug=False,
                   num_devices=NCORES, num_swdge_queues=4,
                   dynamic_dma_scratch_size=65536)

    tabF = nc.dram_tensor("tabF", [R, D], f32, kind="ExternalInput")
    tabRh = nc.dram_tensor("tabRh", [P, KC * FD], bf16, kind="ExternalInput")
    idxh = nc.dram_tensor("idxh", [P, GI], i16, kind="ExternalInput")
    cnth = nc.dram_tensor("cnth", [P, KC * F], bf16, kind="ExternalInput")
    maskh = nc.dram_tensor("maskh", [F, FD], bf16, kind="ExternalInput")
    gbh = nc.dram_tensor("gbh", [2, FD], f32, kind="ExternalInput")
    numh = nc.dram_tensor("numh", [P, NT * N_NUM], f32, kind="ExternalInput")
    out = nc.dram_tensor("out", [BC, OW], f32, kind="ExternalOutput")

    from contextlib import ExitStack
    with ExitStack() as ctx:
        sb = lambda name, shape, dt: ctx.enter_context(
            nc.sbuf_tensor(name, shape, dt))
        idx_sb = sb("idx_sb", [P, GI], i16)
        tabR = sb("tabR", [P, 2 * FD], bf16)  # 2-slot rotating stats chunks
        t2 = sb("t2", [P, 2 * FD], bf16)  # 2-slot rotating squares
        cnt_sb = sb("cnt_sb", [P, KC * F], bf16)
        mask_sb = sb("mask_sb", [F, FD], bf16)
        msk_t = sb("msk_t", [F, FD], f32)
        bufs = [sb(f"buf{k}", [P, FD], f32) for k in range(NP)]
        num_sb = sb("num_sb", [P, NT * N_NUM], f32)
        sc_bc = sb("sc_bc", [P, FD], f32)
        sh_bc = sb("sh_bc", [P, FD], f32)
        ga_sb = sb("ga_sb", [1, FD], f32)[:, :]
        be_sb = sb("be_sb", [1, FD], f32)[:, :]
        ds = sb("ds", [1, FD], f32)[:, :]     # diag sum -> mean
        dq = sb("dq", [1, FD], f32)[:, :]     # diag sumsq -> var -> rstd
        srow = sb("srow", [1, FD], f32)[:, :]  # mean^2 tmp -> scale
        trow = sb("trow", [1, FD], f32)[:, :]  # shift
        ones32 = sb("ones32", [F, 1], f32)
        ones_r = sb("ones_r", [1, P], f32)
        eps_row = sb("eps_row", [1, 1], f32)
        wid_sb = sb("wid_sb", [P, P // 16], i16)

        ps_a = ctx.enter_context(nc.psum_tensor("ps_a", [P, FD], f32))
        ps_b = ctx.enter_context(nc.psum_tensor("ps_b", [P, FD], f32))

        sem = lambda name: ctx.enter_context(nc.semaphore(name))
        s_ld = sem("s_ld")
        s_cnt = sem("s_cnt")
        s_msk = sem("s_msk")
        s_gb2 = sem("s_gb2")
        s_lt = [sem("s_lt0"), sem("s_lt1")]
        s_idx = sem("s_idx")
        s_idxB = sem("s_idxB")
        s_g = [sem(f"s_g{k}") for k in range(NP)]
        s_m = sem("s_m")
        s_v = sem("s_v")
        s_r = sem("s_r")
        s_w = [sem(f"s_w{k}") for k in range(NP)]
        s_num = sem("s_num")
        s_pe = sem("s_pe")
        s_pemq = sem("s_pemq")
        s_sq = sem("s_sq")
        s_tsum = sem("s_tsum")
        s_warm = sem("s_warm")
        s_dve = sem("s_dve")
        s_ax = sem("s_ax")

        # s_ld thresholds after each initial HWDGE load
        LD_CNT, LD_MASK, LD_GA, LD_BE = 16, 32, 48, 64
        LD_TAB = lambda k: 80 + 16 * k  # chunk k loaded

        N_DVE = 12  # s_dve value when sc_bc/sh_bc are ready

        with nc.Block("main") as block:

            @block.sync
            def _(sync):
                H = GI // 2
                sync.dma_start(cnt_sb[:, :], cnth[:, :]).then_inc(s_cnt, 16)
                sync.dma_start(idx_sb[:, 0:H], idxh[:, 0:H]).then_inc(s_idx, 16)
                sync.dma_start(mask_sb[:, :], maskh[:, :]).then_inc(s_msk, 16)
                sync.dma_start(ga_sb, gbh[0:1, :]).then_inc(s_gb2, 16)
                sync.dma_start(be_sb, gbh[1:2, :]).then_inc(s_gb2, 16)
                for k in range(KC):
                    if k >= 2:
                        # slot free when sums-matmuls + square of k-2 done
                        sync.wait_ge(s_tsum, k - 1)
                        sync.wait_ge(s_sq, k - 1)
                    sync.dma_start(
                        tabR[:, (k % 2) * FD:(k % 2 + 1) * FD],
                        tabRh[:, k * FD:(k + 1) * FD],
                    ).then_inc(s_lt[k % 2], 16)
                sync.dma_start(num_sb[:, :], numh[:, :]).then_inc(s_ld, 16)
                sync.dma_start(
                    idx_sb[:, H:2 * H], idxh[:, H:2 * H]).then_inc(s_idxB, 16)
                # numerical columns: per-tile 2-level-AP stores (the 3-level
                # strided form miscompiles on this runtime), interleaved with
                # the embedding stores so their issue cost hides in the
                # inter-tile gaps. s_ld==16 is exact: num is its only load.
                sync.wait_ge(s_ld, 16)
                for t in range(NT):
                    sync.wait_ge(s_r, t + 1)
                    sync.dma_start(
                        out[t * TILE:(t + 1) * TILE, :FD],
                        bufs[t % NP][:, :],
                    ).then_inc(s_w[t % NP], 16)
                    sync.dma_start(
                        out[t * TILE:(t + 1) * TILE, FD:OW],
                        num_sb[:, t * N_NUM:(t + 1) * N_NUM],
                    ).then_inc(s_num, 16)
                for k in range(NP):
                    sync.wait_ge(s_w[k], 16 * ((NT - 1 - k) // NP + 1))
                sync.wait_ge(s_num, 16 * NT)

            @block.gpsimd
            def _(gpsimd):
                for t in range(NT):
                    if t < NT // 2:
                        gpsimd.wait_ge(s_idx, 16)
                    else:
                        gpsimd.wait_ge(s_idxB, 16)
                    if t >= NP:
                        gpsimd.wait_ge(s_w[t % NP], 16 * (t // NP))
                    buf3 = bufs[t % NP][:, :].rearrange(
                        "p (f d) -> p f d", d=D)
                    for q in range(NGPT):
                        g = t * NGPT + q
                        gpsimd.dma_gather(
                            out_ap=buf3[:, q * (F // NGPT):(q + 1) * (F // NGPT), :],
                            in_ap=tabF[:, :],
                            idxs_ap=idx_sb[:, t * GC:(t + 1) * GC],
                            num_idxs=P * (F // NGPT),
                            num_idxs_reg=P * (F // NGPT),
                            elem_size=D,
                            queue_num=q % 4,
                        ).then_inc(s_g[t % NP], 16)

            @block.tensor
            def _(tensor):
                tensor.wait_ge(s_cnt, 16)
                for k in range(KC):
                    tensor.wait_ge(s_lt[k % 2], 16 * (k // 2 + 1))
                    for j in range(4):
                        mm = tensor.matmul(
                            ps_a[0:F, j * 512:(j + 1) * 512],
                            cnt_sb[:, k * F:(k + 1) * F],
                            tabR[:, (k % 2) * FD + j * 512:(k % 2) * FD + (j + 1) * 512],
                            start=(k == 0), stop=(k == KC - 1),
                            skip_group_check=True)
                    if k == KC - 1:
                        mm.then_inc(s_pe, 1)           # s_pe=1: sums done
                    else:
                        mm.then_inc(s_tsum, 1)         # slot consumed by sums
                    tensor.wait_ge(s_sq, k + 1)
                    for j in range(4):
                        mm = tensor.matmul(
                            ps_b[0:F, j * 512:(j + 1) * 512],
                            cnt_sb[:, k * F:(k + 1) * F],
                            t2[:, (k % 2) * FD + j * 512:(k % 2) * FD + (j + 1) * 512],
                            start=(k == 0), stop=(k == KC - 1),
                            skip_group_check=True)
                    mm.then_inc(s_pemq, 1)   # t2 free for chunk k+1; ==8: sq done
                # diag extraction colsums (masked rows live in msk_t[0:F])
                tensor.wait_ge(s_dve, 1)
                for j in range(4):
                    mm = tensor.matmul(
                        ps_a[0:1, j * 512:(j + 1) * 512], ones32[:, :],
                        msk_t[:, j * 512:(j + 1) * 512],
                        start=True, stop=True, skip_group_check=True)
                mm.then_inc(s_pe, 1)                   # s_pe=2: diag_s in ps_a[0]
                tensor.wait_ge(s_dve, 3)
                for j in range(4):
                    mm = tensor.matmul(
                        ps_b[0:1, j * 512:(j + 1) * 512], ones32[:, :],
                        msk_t[:, j * 512:(j + 1) * 512],
                        start=True, stop=True, skip_group_check=True)
                mm.then_inc(s_pe, 1)                   # s_pe=3: diag_q in ps_b[0]
                # broadcast scale/shift rows to 128 partitions
                tensor.wait_ge(s_dve, 8)
                for j in range(4):
                    mm = tensor.matmul(
                        ps_a[:, j * 512:(j + 1) * 512], ones_r[:, :],
                        srow[:, j * 512:(j + 1) * 512],
                        start=True, stop=True, skip_group_check=True)
                mm.then_inc(s_pe, 1)                   # s_pe=4: scale bcast
                tensor.wait_ge(s_dve, 10)
                for j in range(4):
                    mm = tensor.matmul(
                        ps_b[:, j * 512:(j + 1) * 512], ones_r[:, :],
                        trow[:, j * 512:(j + 1) * 512],
                        start=True, stop=True, skip_group_check=True)
                mm.then_inc(s_pe, 1)                   # s_pe=5: shift bcast

            @block.scalar
            def _(scalar):
                for k in range(KC):
                    scalar.wait_ge(s_lt[k % 2], 16 * (k // 2 + 1))
                    if k >= 2:
                        scalar.wait_ge(s_pemq, k - 1)  # t2 slot consumed
                    scalar.square(
                        t2[:, (k % 2) * FD:(k % 2 + 1) * FD],
                        tabR[:, (k % 2) * FD:(k % 2 + 1) * FD],
                    ).then_inc(s_sq, 1)
                # sqrt(var + eps) on the dq row
                scalar.wait_ge(s_dve, 6)
                scalar.activation(
                    dq, dq, mybir.ActivationFunctionType.Sqrt,
                    bias=eps_row[:, :],
                ).then_inc(s_ax, 1)
                # per-tile relu (in place)
                for t in range(NT):
                    scalar.wait_ge(s_v, t + 1)
                    scalar.activation(
                        bufs[t % NP][:, :], bufs[t % NP][:, :],
                        mybir.ActivationFunctionType.Relu,
                    ).then_inc(s_r, 1)

            @block.vector
            def _(vector):
                vector.memset(ones32[:, :], 1.0)
                vector.memset(ones_r[:, :], 1.0)
                vector.memset(eps_row[:, :], float(EPS))  # all 4 partitions
                # ---- stats rows: every DVE op self-chained via s_dve ----
                vsn = [0]

                def vstep(emit, *waits):
                    vector.wait_ge(s_dve, vsn[0])
                    for w_sem, w_val in waits:
                        vector.wait_ge(w_sem, w_val)
                    inst = emit()
                    vsn[0] += 1
                    inst.then_inc(s_dve, 1)
                    return inst

                vstep(lambda: vector.tensor_mul(                    # s_dve=1
                    msk_t[:, :], ps_a[0:F, :], mask_sb[:, :]),
                    (s_pe, 1), (s_msk, 16))
                vstep(lambda: vector.tensor_scalar_mul(
                    ds, ps_a[0:1, :], 1.0 / B),
                    (s_pe, 2))                        # s_dve=2 mean
                vstep(lambda: vector.tensor_mul(                    # s_dve=3
                    msk_t[:, :], ps_b[0:F, :], mask_sb[:, :]),
                    (s_pe, 2), (s_pemq, KC))
                vstep(lambda: vector.tensor_scalar_mul(
                    dq, ps_b[0:1, :], 1.0 / B),
                    (s_pe, 3))                        # s_dve=4 E[x^2]
                vstep(lambda: vector.tensor_mul(
                    srow, ds, ds))                # s_dve=5 mean^2
                vstep(lambda: vector.tensor_sub(
                    dq, dq, srow))                # s_dve=6 var
                vstep(lambda: vector.reciprocal_approx_fast(dq, dq),
                      (s_ax, 1))                                    # s_dve=7 rstd
                vstep(lambda: vector.tensor_mul(
                    srow, ga_sb, dq),
                    (s_gb2, 32))                                  # s_dve=8 scale
                vstep(lambda: vector.tensor_mul(
                    trow, ds, srow))              # s_dve=9
                vstep(lambda: vector.tensor_sub(
                    trow, be_sb, trow),
                    (s_gb2, 32))                                  # s_dve=10 shift
                vstep(lambda: vector.tensor_copy(sc_bc[:, :], ps_a[:, :]),
                      (s_pe, 4))                                    # s_dve=11
                vstep(lambda: vector.tensor_copy(sh_bc[:, :], ps_b[:, :]),
                      (s_pe, 5))                                    # s_dve=12
                assert vsn[0] == N_DVE
                # ---- per-tile normalize (in place) ----
                for t in range(NT):
                    vector.wait_ge(s_dve, N_DVE)
                    vector.wait_ge(s_g[t % NP], 16 * NGPT * (t // NP + 1))
                    vector.tensor_mul(
                        bufs[t % NP][:, :], bufs[t % NP][:, :],
                        sc_bc[:, :]).then_inc(s_m, 1)
                    vector.wait_ge(s_m, t + 1)
                    vector.tensor_add(
                        bufs[t % NP][:, :], bufs[t % NP][:, :],
                        sh_bc[:, :]).then_inc(s_v, 1)

        nc.compile()
    return nc


_NC_CACHE: list = []

# Optional profiling knobs (used by test harnesses; harmless defaults).
TRACE = False
TMPDIR = None
LAST_RESULT: list = []


def _get_nc():
    if not _NC_CACHE:
        _NC_CACHE.append(_build_nc())
    return _NC_CACHE[0]


def _host_prep(cat_idx, numerical, tables, gamma, beta):
    """Host-side layout/preprocessing (indices + replication only)."""
    # linear gather rows: row = v*F + f  (max 32031, fits int16)
    lin = (cat_idx.astype(np.int32) * F
           + np.arange(F, dtype=np.int32)[None, :])          # [B, F]

    # dma_gather idx layout. Gather g = t*NGPT + q covers logical indices
    # i = f_local*128 + p (f = q*8 + f_local, batch row = t*128 + p); the
    # ucode reads logical index i from partition i%16, column g*GC + i//16,
    # replicated to all 8 16-partition groups.
    i_grid = np.arange(P * (F // NGPT))                       # [1024]
    f_loc = i_grid >> 7                                       # i // 128
    p_ = i_grid & 127                                         # i % 128
    ip = (i_grid % 16)                                        # partition in 16
    jc = (i_grid // 16)                                       # column in gather
    lin16 = lin.astype(np.int16).reshape(NCORES, NT, TILE, NGPT, F // NGPT)
    # Queue q's Q7 pair (cores 2q, 2q+1) reads only partitions 32q..32q+31;
    # both 16-partition halves need the same wrapped block. Column t*GC+jc.
    idx_pc = np.zeros((NCORES, 128, GI), dtype=np.int16)
    col = np.arange(NT)[:, None] * GC + jc[None, :]           # [NT, 1024]
    iprep = ip[None, :].repeat(NT, 0)                         # [NT, 1024]
    for q in range(NGPT):
        vals = lin16[:, :, :, q, :][:, :, p_, f_loc]          # [NC, NT, 1024]
        idx_pc[:, 32 * q + iprep, col] = vals
        idx_pc[:, 32 * q + 16 + iprep, col] = vals
    idx_pc = np.ascontiguousarray(idx_pc)

    # global histogram over linear rows (integer-only preprocessing)
    cnt = np.bincount(lin.ravel(), minlength=VP * F).reshape(VP, F)
    cnt_in = np.ascontiguousarray(
        cnt.reshape(KC, TILE, F).transpose(1, 0, 2).reshape(P, KC * F)
    ).astype(ml_dtypes.bfloat16)

    # flat gather table [R, D] f32, row = v*F + f
    tabF = np.ascontiguousarray(
        tables.transpose(1, 0, 2).reshape(R, D)).astype(np.float32)

    # stats table layout [128, k*2048 + f*64 + d] = T[f, k*128+p, d], bf16
    tpad = np.zeros((F, VP, D), dtype=np.float32)
    tpad[:, :V] = tables
    tabR = np.ascontiguousarray(
        tpad.reshape(F, KC, TILE, D).transpose(2, 1, 0, 3).reshape(P, KC * FD)
    ).astype(ml_dtypes.bfloat16)

    mask = np.zeros((F, FD), dtype=np.float32)
    for f in range(F):
        mask[f, f * D:(f + 1) * D] = 1.0
    mask = mask.astype(ml_dtypes.bfloat16)

    gb = np.ascontiguousarray(
        np.stack([gamma.reshape(FD), beta.reshape(FD)], axis=0))

    num_pc = []
    for c in range(NCORES):
        sh = numerical[c * BC:(c + 1) * BC].reshape(NT, P, N_NUM)
        num_pc.append(np.ascontiguousarray(
            sh.transpose(1, 0, 2).reshape(P, NT * N_NUM)))

    return idx_pc, cnt_in, tabF, tabR, mask, gb, num_pc


def kernel(cat_idx, numerical, tables, gamma, beta):
    cat_idx = np.asarray(cat_idx)
    numerical = np.asarray(numerical, dtype=np.float32)
    tables = np.asarray(tables, dtype=np.float32)
    gamma = np.asarray(gamma, dtype=np.float32)
    beta = np.asarray(beta, dtype=np.float32)

    nc = _get_nc()
    idx_pc, cnt_in, tabF, tabR, mask, gb, num_pc = _host_prep(
        cat_idx, numerical, tables, gamma, beta)

    in_maps = [
        {"tabF": tabF, "tabRh": tabR, "idxh": idx_pc[c], "cnth": cnt_in,
         "maskh": mask, "gbh": gb, "numh": num_pc[c]}
        for c in range(NCORES)
    ]
    res = run_bass_kernel_spmd(nc, in_maps, core_ids=list(range(NCORES)),
                               trace=TRACE, tmpdir=TMPDIR)
    LAST_RESULT.clear()
    LAST_RESULT.append(res)
    out = np.concatenate([res.results[c]["out"] for c in range(NCORES)], axis=0)
    return out



# revision 36
# speedup vs baseline: 1.0692x; 1.0146x over previous
"""Trainium2 Bass kernel for CategoricalDnn: embedding gather + BatchNorm(train) + ReLU + concat.

Reference computation (B=65536, F=32, V=1001, D=64, N_NUM=16):
    emb[b,f,:]  = tables[f, cat_idx[b,f], :]
    mean/var    = biased batch stats of emb over b (global batch)
    normed      = (emb - mean) * rsqrt(var+eps) * gamma + beta
    out         = concat([relu(normed).reshape(B, F*D), numerical], axis=1)

Strategy (8 NeuronCores, data-parallel over the batch):
  * Batch stats do not need the gathered data: sum_b T[f,idx_b,d] =
    sum_v count[f,v]*T[f,v,d]. The host computes the GLOBAL index histogram
    (integer-only preprocessing of cat_idx); each core contracts it against
    the table on the PE (count^T @ [T | T^2] over v-chunks of 128) to get
    identical global statistics -- no collective, no second data pass.
  * Single gather pass via the ant-ucode dma_gather: 1024 rows x 256B per
    instruction (the SWDGE descriptor ring caps one instruction at 1024
    descriptors; 2048 crashes the runtime). 4 gathers cover one 128-row
    tile across all 32 features -- 8x fewer SWDGE instructions than the
    per-feature indirect-DMA form, which was issue-bound on gpsimd at
    ~1.4us per instruction.
  * Per tile, in a rotating buffer pool: DVE x*scale+shift in place,
    ACT ReLU in place, HWDGE store. Numerical columns are stored once by a
    single strided DMA.
"""

import sys

import numpy as np

if "/opt/trn_rl_repo" not in sys.path:
    sys.path.insert(0, "/opt/trn_rl_repo")

import ml_dtypes

import concourse.bacc as bacc
import concourse.bass as bass
import concourse.mybir as mybir
from concourse.bass_utils import run_bass_kernel_spmd

# Problem constants (hardcoded per harness contract).
B, F, V, D, N_NUM = 65536, 32, 1001, 64, 16
EPS = 1e-5
NCORES = 8
BC = B // NCORES          # 8192 batch rows per core
TILE = 128                # batch rows per gather tile
NT = BC // TILE           # 64 tiles per core
FD = F * D                # 2048
OW = FD + N_NUM           # 2064 output columns
R = F * V                 # 32032 flat table rows (row = v*F + f)
VP = 1024                 # padded vocab (8 chunks of 128)
KC = VP // 128            # 8 v-chunks
P = 128
NGPT = 4                  # dma_gather calls per tile (1024 idxs each)
NG = NT * NGPT            # gathers per core
GC = P * (F // NGPT) // 16  # idx columns per gather
GI = NT * GC              # idx columns per partition (int16; per-queue data in its pair's partitions)

f32 = mybir.dt.float32
bf16 = mybir.dt.bfloat16
i32 = mybir.dt.int32
i16 = mybir.dt.int16

NP = 6                    # rotating tile buffers


def _build_nc() -> bass.Bass:
    nc = bacc.Bacc("TRN2", target_bir_lowering=False, debug=False,
                   num_devices=NCORES, num_swdge_queues=4,
                   dynamic_dma_scratch_size=65536)

    tabF = nc.dram_tensor("tabF", [R, D], f32, kind="ExternalInput")
    tabRh = nc.dram_tensor("tabRh", [P, KC * FD], bf16, kind="ExternalInput")
    idxh = nc.dram_tensor("idxh", [P, GI], i16, kind="ExternalInput")
    cnth = nc.dram_tensor("cnth", [P, KC * F], bf16, kind="ExternalInput")
    maskh = nc.dram_tensor("maskh", [F, FD], bf16, kind="ExternalInput")
    gbh = nc.dram_tensor("gbh", [2, FD], f32, kind="ExternalInput")
    numh = nc.dram_tensor("numh", [P, NT * N_NUM], f32, kind="ExternalInput")
    out = nc.dram_tensor("out", [BC, OW], f32, kind="ExternalOutput")
    rowsc = nc.dram_tensor("rowsc", [2, FD], f32, kind="ExternalOutput")

    from contextlib import ExitStack
    with ExitStack() as ctx:
        sb = lambda name, shape, dt: ctx.enter_context(
            nc.sbuf_tensor(name, shape, dt))
        idx_sb = sb("idx_sb", [P, GI], i16)
        tabR = sb("tabR", [P, 2 * FD], bf16)  # 2-slot rotating stats chunks
        t2 = sb("t2", [P, 2 * FD], bf16)  # 2-slot rotating squares
        cnt_sb = sb("cnt_sb", [P, KC * F], bf16)
        mask_sb = sb("mask_sb", [F, FD], bf16)
        msk_t = sb("msk_t", [F, FD], f32)
        bufs = [sb(f"buf{k}", [P, FD], f32) for k in range(NP)]
        num_sb = sb("num_sb", [P, NT * N_NUM], f32)
        sc_bc = sb("sc_bc", [P, FD], f32)
        sh_bc = sb("sh_bc", [P, FD], f32)
        ga_sb = sb("ga_sb", [1, FD], f32)[:, :]
        be_sb = sb("be_sb", [1, FD], f32)[:, :]
        ds = sb("ds", [1, FD], f32)[:, :]     # diag sum -> mean
        dq = sb("dq", [1, FD], f32)[:, :]     # diag sumsq -> var -> rstd
        srow = sb("srow", [1, FD], f32)[:, :]  # mean^2 tmp -> scale
        trow = sb("trow", [1, FD], f32)[:, :]  # shift
        ones32 = sb("ones32", [F, 1], f32)
        eps_row = sb("eps_row", [1, 1], f32)
        wid_sb = sb("wid_sb", [P, P // 16], i16)

        ps_a = ctx.enter_context(nc.psum_tensor("ps_a", [P, FD], f32))
        ps_b = ctx.enter_context(nc.psum_tensor("ps_b", [P, FD], f32))

        sem = lambda name: ctx.enter_context(nc.semaphore(name))
        s_ld = sem("s_ld")
        s_cnt = sem("s_cnt")
        s_msk = sem("s_msk")
        s_gb2 = sem("s_gb2")
        s_lt = [sem("s_lt0"), sem("s_lt1")]
        s_idx = sem("s_idx")
        s_idxB = sem("s_idxB")
        s_g = [sem(f"s_g{k}") for k in range(NP)]
        s_m = sem("s_m")
        s_v = sem("s_v")
        s_r = sem("s_r")
        s_w = [sem(f"s_w{k}") for k in range(NP)]
        s_num = sem("s_num")
        s_pe = sem("s_pe")
        s_pemq = sem("s_pemq")
        s_sq = sem("s_sq")
        s_tsum = sem("s_tsum")
        s_warm = sem("s_warm")
        s_bc = sem("s_bc")
        s_rowd = sem("s_rowd")
        s_dve = sem("s_dve")
        s_ax = sem("s_ax")

        # s_ld thresholds after each initial HWDGE load
        LD_CNT, LD_MASK, LD_GA, LD_BE = 16, 32, 48, 64
        LD_TAB = lambda k: 80 + 16 * k  # chunk k loaded

        N_DVE = 10  # s_dve value when srow/trow are final

        with nc.Block("main") as block:

            @block.sync
            def _(sync):
                H = GI // 2
                sync.dma_start(cnt_sb[:, :], cnth[:, :]).then_inc(s_cnt, 16)
                sync.dma_start(idx_sb[:, 0:H], idxh[:, 0:H]).then_inc(s_idx, 16)
                sync.dma_start(mask_sb[:, :], maskh[:, :]).then_inc(s_msk, 16)
                sync.dma_start(ga_sb, gbh[0:1, :]).then_inc(s_gb2, 16)
                sync.dma_start(be_sb, gbh[1:2, :]).then_inc(s_gb2, 16)
                for k in range(KC):
                    if k >= 2:
                        # slot free when sums-matmuls + square of k-2 done
                        sync.wait_ge(s_tsum, k - 1)
                        sync.wait_ge(s_sq, k - 1)
                    sync.dma_start(
                        tabR[:, (k % 2) * FD:(k % 2 + 1) * FD],
                        tabRh[:, k * FD:(k + 1) * FD],
                    ).then_inc(s_lt[k % 2], 16)
                sync.dma_start(num_sb[:, :], numh[:, :]).then_inc(s_ld, 16)
                sync.dma_start(
                    idx_sb[:, H:2 * H], idxh[:, H:2 * H]).then_inc(s_idxB, 16)
                # numerical columns: per-tile 2-level-AP stores (the 3-level
                # strided form miscompiles on this runtime), interleaved with
                # the embedding stores so their issue cost hides in the
                # inter-tile gaps. s_ld==16 is exact: num is its only load.
                # broadcast scale/shift rows to all 128 partitions via a
                # DRAM bounce + stride-0-source load (replaces PE broadcast)
                sync.wait_ge(s_dve, 8)
                sync.dma_start(rowsc[0:1, :], srow).then_inc(s_rowd, 16)
                sync.wait_ge(s_dve, 10)
                sync.dma_start(rowsc[1:2, :], trow).then_inc(s_rowd, 16)
                sync.wait_ge(s_rowd, 32)
                sync.dma_start(
                    sc_bc[:, :],
                    rowsc[0:1, :].partition_broadcast(P).squeeze(1),
                ).then_inc(s_bc, 16)
                sync.dma_start(
                    sh_bc[:, :],
                    rowsc[1:2, :].partition_broadcast(P).squeeze(1),
                ).then_inc(s_bc, 16)
                sync.wait_ge(s_ld, 16)
                for t in range(NT):
                    sync.wait_ge(s_r, t + 1)
                    sync.dma_start(
                        out[t * TILE:(t + 1) * TILE, :FD],
                        bufs[t % NP][:, :],
                    ).then_inc(s_w[t % NP], 16)
                    sync.dma_start(
                        out[t * TILE:(t + 1) * TILE, FD:OW],
                        num_sb[:, t * N_NUM:(t + 1) * N_NUM],
                    ).then_inc(s_num, 16)
                for k in range(NP):
                    sync.wait_ge(s_w[k], 16 * ((NT - 1 - k) // NP + 1))
                sync.wait_ge(s_num, 16 * NT)

            @block.gpsimd
            def _(gpsimd):
                for t in range(NT):
                    if t < NT // 2:
                        gpsimd.wait_ge(s_idx, 16)
                    else:
                        gpsimd.wait_ge(s_idxB, 16)
                    if t >= NP:
                        gpsimd.wait_ge(s_w[t % NP], 16 * (t // NP))
                    buf3 = bufs[t % NP][:, :].rearrange(
                        "p (f d) -> p f d", d=D)
                    for q in range(NGPT):
                        g = t * NGPT + q
                        gpsimd.dma_gather(
                            out_ap=buf3[:, q * (F // NGPT):(q + 1) * (F // NGPT), :],
                            in_ap=tabF[:, :],
                            idxs_ap=idx_sb[:, t * GC:(t + 1) * GC],
                            num_idxs=P * (F // NGPT),
                            num_idxs_reg=P * (F // NGPT),
                            elem_size=D,
                            queue_num=q % 4,
                        ).then_inc(s_g[t % NP], 16)

            @block.tensor
            def _(tensor):
                tensor.wait_ge(s_cnt, 16)
                for k in range(KC):
                    tensor.wait_ge(s_lt[k % 2], 16 * (k // 2 + 1))
                    for j in range(4):
                        mm = tensor.matmul(
                            ps_a[0:F, j * 512:(j + 1) * 512],
                            cnt_sb[:, k * F:(k + 1) * F],
                            tabR[:, (k % 2) * FD + j * 512:(k % 2) * FD + (j + 1) * 512],
                            start=(k == 0), stop=(k == KC - 1),
                            skip_group_check=True)
                    if k == KC - 1:
                        mm.then_inc(s_pe, 1)           # s_pe=1: sums done
                    else:
                        mm.then_inc(s_tsum, 1)         # slot consumed by sums
                    tensor.wait_ge(s_sq, k + 1)
                    for j in range(4):
                        mm = tensor.matmul(
                            ps_b[0:F, j * 512:(j + 1) * 512],
                            cnt_sb[:, k * F:(k + 1) * F],
                            t2[:, (k % 2) * FD + j * 512:(k % 2) * FD + (j + 1) * 512],
                            start=(k == 0), stop=(k == KC - 1),
                            skip_group_check=True)
                    mm.then_inc(s_pemq, 1)   # t2 free for chunk k+1; ==8: sq done
                # diag extraction colsums (masked rows live in msk_t[0:F])
                tensor.wait_ge(s_dve, 1)
                for j in range(4):
                    mm = tensor.matmul(
                        ps_a[0:1, j * 512:(j + 1) * 512], ones32[:, :],
                        msk_t[:, j * 512:(j + 1) * 512],
                        start=True, stop=True, skip_group_check=True)
                mm.then_inc(s_pe, 1)                   # s_pe=2: diag_s in ps_a[0]
                tensor.wait_ge(s_dve, 3)
                for j in range(4):
                    mm = tensor.matmul(
                        ps_b[0:1, j * 512:(j + 1) * 512], ones32[:, :],
                        msk_t[:, j * 512:(j + 1) * 512],
                        start=True, stop=True, skip_group_check=True)
                mm.then_inc(s_pe, 1)                   # s_pe=3: diag_q in ps_b[0]

            @block.scalar
            def _(scalar):
                for k in range(KC):
                    scalar.wait_ge(s_lt[k % 2], 16 * (k // 2 + 1))
                    if k >= 2:
                        scalar.wait_ge(s_pemq, k - 1)  # t2 slot consumed
                    scalar.square(
                        t2[:, (k % 2) * FD:(k % 2 + 1) * FD],
                        tabR[:, (k % 2) * FD:(k % 2 + 1) * FD],
                    ).then_inc(s_sq, 1)
                # sqrt(var + eps) on the dq row
                scalar.wait_ge(s_dve, 6)
                scalar.activation(
                    dq, dq, mybir.ActivationFunctionType.Sqrt,
                    bias=eps_row[:, :],
                ).then_inc(s_ax, 1)
                # per-tile relu (in place)
                for t in range(NT):
                    scalar.wait_ge(s_v, t + 1)
                    scalar.activation(
                        bufs[t % NP][:, :], bufs[t % NP][:, :],
                        mybir.ActivationFunctionType.Relu,
                    ).then_inc(s_r, 1)

            @block.vector
            def _(vector):
                vector.memset(ones32[:, :], 1.0)
                vector.memset(eps_row[:, :], float(EPS))  # all 4 partitions
                # ---- stats rows: every DVE op self-chained via s_dve ----
                vsn = [0]

                def vstep(emit, *waits):
                    vector.wait_ge(s_dve, vsn[0])
                    for w_sem, w_val in waits:
                        vector.wait_ge(w_sem, w_val)
                    inst = emit()
                    vsn[0] += 1
                    inst.then_inc(s_dve, 1)
                    return inst

                vstep(lambda: vector.tensor_mul(                    # s_dve=1
                    msk_t[:, :], ps_a[0:F, :], mask_sb[:, :]),
                    (s_pe, 1), (s_msk, 16))
                vstep(lambda: vector.tensor_scalar_mul(
                    ds, ps_a[0:1, :], 1.0 / B),
                    (s_pe, 2))                        # s_dve=2 mean
                vstep(lambda: vector.tensor_mul(                    # s_dve=3
                    msk_t[:, :], ps_b[0:F, :], mask_sb[:, :]),
                    (s_pe, 2), (s_pemq, KC))
                vstep(lambda: vector.tensor_scalar_mul(
                    dq, ps_b[0:1, :], 1.0 / B),
                    (s_pe, 3))                        # s_dve=4 E[x^2]
                vstep(lambda: vector.tensor_mul(
                    srow, ds, ds))                # s_dve=5 mean^2
                vstep(lambda: vector.tensor_sub(
                    dq, dq, srow))                # s_dve=6 var
                vstep(lambda: vector.reciprocal_approx_fast(dq, dq),
                      (s_ax, 1))                                    # s_dve=7 rstd
                vstep(lambda: vector.tensor_mul(
                    srow, ga_sb, dq),
                    (s_gb2, 32))                                  # s_dve=8 scale
                vstep(lambda: vector.tensor_mul(
                    trow, ds, srow))              # s_dve=9
                vstep(lambda: vector.tensor_sub(
                    trow, be_sb, trow),
                    (s_gb2, 32))                                  # s_dve=10 shift
                assert vsn[0] == N_DVE
                # ---- per-tile normalize (in place) ----
                for t in range(NT):
                    vector.wait_ge(s_bc, 32)
                    vector.wait_ge(s_g[t % NP], 16 * NGPT * (t // NP + 1))
                    vector.tensor_mul(
                        bufs[t % NP][:, :], bufs[t % NP][:, :],
                        sc_bc[:, :]).then_inc(s_m, 1)
                    vector.wait_ge(s_m, t + 1)
                    vector.tensor_add(
                        bufs[t % NP][:, :], bufs[t % NP][:, :],
                        sh_bc[:, :]).then_inc(s_v, 1)

        nc.compile()
    return nc


_NC_CACHE: list = []

# Optional profiling knobs (used by test harnesses; harmless defaults).
TRACE = False
TMPDIR = None
LAST_RESULT: list = []


def _get_nc():
    if not _NC_CACHE:
        _NC_CACHE.append(_build_nc())
    return _NC_CACHE[0]


def _host_prep(cat_idx, numerical, tables, gamma, beta):
    """Host-side layout/preprocessing (indices + replication only)."""
    # linear gather rows: row = v*F + f  (max 32031, fits int16)
    lin = (cat_idx.astype(np.int32) * F
           + np.arange(F, dtype=np.int32)[None, :])          # [B, F]

    # dma_gather idx layout. Gather g = t*NGPT + q covers logical indices
    # i = f_local*128 + p (f = q*8 + f_local, batch row = t*128 + p); the
    # ucode reads logical index i from partition i%16, column g*GC + i//16,
    # replicated to all 8 16-partition groups.
    i_grid = np.arange(P * (F // NGPT))                       # [1024]
    f_loc = i_grid >> 7                                       # i // 128
    p_ = i_grid & 127                                         # i % 128
    ip = (i_grid % 16)                                        # partition in 16
    jc = (i_grid // 16)                                       # column in gather
    lin16 = lin.astype(np.int16).reshape(NCORES, NT, TILE, NGPT, F // NGPT)
    # Queue q's Q7 pair (cores 2q, 2q+1) reads only partitions 32q..32q+31;
    # both 16-partition halves need the same wrapped block. Column t*GC+jc.
    idx_pc = np.zeros((NCORES, 128, GI), dtype=np.int16)
    col = np.arange(NT)[:, None] * GC + jc[None, :]           # [NT, 1024]
    iprep = ip[None, :].repeat(NT, 0)                         # [NT, 1024]
    for q in range(NGPT):
        vals = lin16[:, :, :, q, :][:, :, p_, f_loc]          # [NC, NT, 1024]
        idx_pc[:, 32 * q + iprep, col] = vals
        idx_pc[:, 32 * q + 16 + iprep, col] = vals
    idx_pc = np.ascontiguousarray(idx_pc)

    # global histogram over linear rows (integer-only preprocessing)
    cnt = np.bincount(lin.ravel(), minlength=VP * F).reshape(VP, F)
    cnt_in = np.ascontiguousarray(
        cnt.reshape(KC, TILE, F).transpose(1, 0, 2).reshape(P, KC * F)
    ).astype(ml_dtypes.bfloat16)

    # flat gather table [R, D] f32, row = v*F + f
    tabF = np.ascontiguousarray(
        tables.transpose(1, 0, 2).reshape(R, D)).astype(np.float32)

    # stats table layout [128, k*2048 + f*64 + d] = T[f, k*128+p, d], bf16
    tpad = np.zeros((F, VP, D), dtype=np.float32)
    tpad[:, :V] = tables
    tabR = np.ascontiguousarray(
        tpad.reshape(F, KC, TILE, D).transpose(2, 1, 0, 3).reshape(P, KC * FD)
    ).astype(ml_dtypes.bfloat16)

    mask = np.zeros((F, FD), dtype=np.float32)
    for f in range(F):
        mask[f, f * D:(f + 1) * D] = 1.0
    mask = mask.astype(ml_dtypes.bfloat16)

    gb = np.ascontiguousarray(
        np.stack([gamma.reshape(FD), beta.reshape(FD)], axis=0))

    num_pc = []
    for c in range(NCORES):
        sh = numerical[c * BC:(c + 1) * BC].reshape(NT, P, N_NUM)
        num_pc.append(np.ascontiguousarray(
            sh.transpose(1, 0, 2).reshape(P, NT * N_NUM)))

    return idx_pc, cnt_in, tabF, tabR, mask, gb, num_pc


def kernel(cat_idx, numerical, tables, gamma, beta):
    cat_idx = np.asarray(cat_idx)
    numerical = np.asarray(numerical, dtype=np.float32)
    tables = np.asarray(tables, dtype=np.float32)
    gamma = np.asarray(gamma, dtype=np.float32)
    beta = np.asarray(beta, dtype=np.float32)

    nc = _get_nc()
    idx_pc, cnt_in, tabF, tabR, mask, gb, num_pc = _host_prep(
        cat_idx, numerical, tables, gamma, beta)

    in_maps = [
        {"tabF": tabF, "tabRh": tabR, "idxh": idx_pc[c], "cnth": cnt_in,
         "maskh": mask, "gbh": gb, "numh": num_pc[c]}
        for c in range(NCORES)
    ]
    res = run_bass_kernel_spmd(nc, in_maps, core_ids=list(range(NCORES)),
                               trace=TRACE, tmpdir=TMPDIR)
    LAST_RESULT.clear()
    LAST_RESULT.append(res)
    out = np.concatenate([res.results[c]["out"] for c in range(NCORES)], axis=0)
    return out

